# revision 23
# baseline (speedup 1.0000x reference)
"""CrossScaleAttention Trainium2 kernel.

Full inputs -> full output; shards batch (8 samples) across 8 NeuronCores,
one sample per core (pure data parallel, replicated weights).

Per-core algorithm (restructured from the reference; validated in numpy):
  - bilinear 1/3 downsample with align_corners=False == exact subsample at
    (3i+1, 3j+1).
  - score computed directly in [l, p] layout: stationary operands are the
    normalized ref-patch tap weights (scaled by 10/max(||patch||, eps)),
    moving operand is a 52-wide padded match plane stacked two-tap-deep on
    partitions (dy=0 upper / dy=1 lower halves), so each position-block is
    one contiguous free-dim run. 6 matmuls per (l-half, 8-row block).
  - softmax over l is max-free (empirical |logit| <= ~69 < 88 overflow):
    exp straight out of PSUM, partition-dim sums via ones-matmul, scale by
    a broadcast reciprocal. No PE transposes needed: [l, p] is already the
    layout the deconv wants.
  - dynamic transposed conv decomposes into 9 residue grids (rh, rw); each is
    an accumulation over 18 units (m, n, l-chunk) of matmuls
    S_r[c, ji] += G_{r,m,n}[l, c].T @ attn_shift[l, ji], where
    G_{r,m,n}[l, c] = prelu(gather(x_pad) @ wa^T) is produced directly by the
    TensorEngine from a strided gather of padded x (fusing conv_assembly).
    attn shifts are flat offsets into the 50-wide zero-padded attn plane, so
    the matmul rhs stays a single contiguous free-dim run (N = nj*50).
  - residue planes are interleaved into full output rows in SBUF, DMA'd out.
"""

import sys

for _p in ("/opt/trn_rl_repo", "/root/.axon_site/_ro/trn_rl_repo"):
    if _p not in sys.path:
        sys.path.append(_p)

import numpy as np

import concourse.bass as bass
import concourse.tile as tile
from concourse import bacc, mybir
from concourse.bass_utils import run_bass_kernel_spmd
from concourse.masks import make_identity

F32 = mybir.dt.float32
F32R = mybir.dt.float32r
B16 = mybir.dt.bfloat16
AF = mybir.ActivationFunctionType

# Problem constants (hardcoded per contract)
B, C, H, W = 8, 128, 48, 48
CM = 64
HL = WL = 16
L = 256               # reference patches
SM = 10.0
ESC = 1.0e-4
HP = H + 6            # x padded by 3 -> 54
M2H, M2W = 53, 52     # match plane: 50 padded rows (+3 guard), 52-wide rows
APH, APW = 51, 50     # attn pad: rows 0..49 + guard row, 50-wide
NB = 6                # phase-B position blocks (8 output rows each)
NBR = 8               # rows per phase-B block
NBN = NBR * M2W       # moving-run length per phase-B matmul (416)
JBS = [10, 10, 10, 10, 8]   # j-row blocks for the tconv
STRIDED_TCONV = False  # 48-wide strided tconv rhs (N=480) vs flat 50-wide


def build_program(a1, a2, aa, need_ba, use_prelu=True,
                  mm_dt=B16, score_dt=F32R):
    """Build the single-core Bass program. Scalars a1/a2/aa baked as imms."""
    nc = bacc.Bacc("TRN2", target_bir_lowering=False, debug=False)

    x = nc.dram_tensor("x", [C, H, W], F32, kind="ExternalInput").ap()
    w1t = nc.dram_tensor("w1t", [C, CM], F32, kind="ExternalInput").ap()
    w2t = nc.dram_tensor("w2t", [C, CM], F32, kind="ExternalInput").ap()
    wat = nc.dram_tensor("wat", [C, C], F32, kind="ExternalInput").ap()
    b1 = nc.dram_tensor("b1", [CM, 1], F32, kind="ExternalInput").ap()
    b2 = nc.dram_tensor("b2", [CM, 1], F32, kind="ExternalInput").ap()
    bar = nc.dram_tensor("bar", [1, C], F32, kind="ExternalInput").ap()
    out = nc.dram_tensor("out", [C, 3 * H, 3 * W], F32, kind="ExternalOutput").ap()

    def prelu_evac(out_ap, in_ap, alpha, bias=0.0):
        nc.scalar.activation(out_ap, in_ap, AF.Prelu,
                             bias=bias, scale=1.0, alpha=float(alpha))

    with tile.TileContext(nc) as tc:
        import contextlib
        ctx = contextlib.ExitStack()
        with ctx:
            consts = ctx.enter_context(tc.tile_pool(name="consts", bufs=1))
            work = ctx.enter_context(tc.tile_pool(name="work", bufs=3))
            small = ctx.enter_context(tc.tile_pool(name="small", bufs=4))
            gpool = ctx.enter_context(tc.tile_pool(name="gpool", bufs=4))
            stpool = ctx.enter_context(tc.tile_pool(name="stage", bufs=2))
            ps_acc = ctx.enter_context(
                tc.tile_pool(name="ps_acc", bufs=4, space="PSUM"))
            ps_aux = ctx.enter_context(
                tc.tile_pool(name="ps_aux", bufs=3, space="PSUM"))

            # ---- constants / inputs in SBUF ----
            w1t_sb = consts.tile([C, CM], F32)
            w2t_sb = consts.tile([C, CM], F32)
            wat_sb = consts.tile([C, C], F32)
            b1_sb = consts.tile([CM, 1], F32)
            b2_sb = consts.tile([CM, 1], F32)
            ones64 = consts.tile([CM, 1], F32)
            ones1 = consts.tile([1, C], F32)
            onesq = consts.tile([C, C], score_dt)
            negk = consts.tile([C, 1], F32)
            ident = consts.tile([C, C], F32)
            x_sb = consts.tile([C, H * W], F32)
            xpad = consts.tile([C, HP, HP], F32)
            mpad2 = consts.tile([C, M2H, M2W], score_dt)
            mpad3 = consts.tile([C, M2H, M2W], score_dt)
            rpad = consts.tile([CM, 18, 18], F32)
            xsub = consts.tile([C, L], F32)
            wat_bf = consts.tile([C, C], B16)
            wnp = [consts.tile([C, L], score_dt, name=f"wnp{i}",
                               tag=f"wnp{i}") for i in range(3)]
            wnq = consts.tile([C, L], score_dt)
            wns2 = consts.tile([CM, L], score_dt)
            invb = consts.tile([C, L], F32)
            esb = [consts.tile([C, NB * 8 * 48], score_dt, name=f"esb{i}",
                               tag=f"esb{i}") for i in range(2)]
            apad = [consts.tile([C, APH, APW], mm_dt, name=f"apad{i}",
                                tag=f"apad{i}") for i in range(2)]
            if need_ba:
                bar_sb = consts.tile([1, C], F32)
                nc.sync.dma_start(bar_sb[:], bar)

            nc.sync.dma_start(w1t_sb[:], w1t)
            nc.sync.dma_start(w2t_sb[:], w2t)
            nc.sync.dma_start(wat_sb[:], wat)
            nc.sync.dma_start(b1_sb[:], b1)
            nc.sync.dma_start(b2_sb[:], b2)
            for j0 in range(0, 48, 10):
                nj = min(10, 48 - j0)
                nc.sync.dma_start(
                    x_sb[:, j0 * 48:(j0 + nj) * 48],
                    x.rearrange("c h w -> c (h w)")[:, j0 * 48:(j0 + nj) * 48])
            nc.sync.dma_start(xpad[:, 3:51, 3:51], x)

            nc.gpsimd.memset(ones64[:], 1.0)
            nc.gpsimd.memset(ones1[:], 1.0)
            nc.vector.memset(onesq[:].bitcast(F32), 1.0)
            nc.gpsimd.memset(negk[:], -40.0)
            make_identity(nc, ident[:])

            # HAM warm-up: ~40 throwaway matmuls on the identity while the
            # input DMAs land, so real matmuls start at K=8/8.
            wps = ps_aux.tile([C, 512], F32, tag="aux")
            for wi in range(40):
                nc.tensor.matmul(wps[:, :C], ident[:], ident[:],
                                 start=True, stop=True)

            nc.vector.tensor_copy(wat_bf[:], wat_sb[:])
            # x -> xpad interior; zero borders (pad 3)
            nc.gpsimd.memset(xpad[:, 0:3, :], 0.0)
            nc.gpsimd.memset(xpad[:, 51:54, :], 0.0)
            nc.gpsimd.memset(xpad[:, 3:51, 0:3], 0.0)
            nc.gpsimd.memset(xpad[:, 3:51, 51:54], 0.0)

            # match plane zeros: upper half holds padded rows 0..49, lower
            # half the same plane shifted up one row; cols 0/49 and the
            # 50..51 guard columns are zero, plus guard rows.
            nc.vector.memset(mpad2[0:64, 0, :].bitcast(F32), 0.0)
            nc.vector.memset(mpad2[0:64, 49:53, :].bitcast(F32), 0.0)
            nc.vector.memset(mpad2[0:64, 1:49, 0].bitcast(F32), 0.0)
            nc.vector.memset(mpad2[0:64, 1:49, 49:52].bitcast(F32), 0.0)
            nc.vector.memset(mpad2[64:128, 48:53, :].bitcast(F32), 0.0)
            nc.vector.memset(mpad2[64:128, 0:48, 0].bitcast(F32), 0.0)
            nc.vector.memset(mpad2[64:128, 0:48, 49:52].bitcast(F32), 0.0)
            # mpad3: upper = P shifted up 2 rows; lower = up 2 rows + left
            # 1 col (dy=2 tap pairs for dx=0,1; dx=2 single reads upper+2)
            nc.vector.memset(mpad3[0:64, 47:53, :].bitcast(F32), 0.0)
            nc.vector.memset(mpad3[0:64, 0:47, 0].bitcast(F32), 0.0)
            nc.vector.memset(mpad3[0:64, 0:47, 49:52].bitcast(F32), 0.0)
            nc.vector.memset(mpad3[64:128, 47:53, :].bitcast(F32), 0.0)
            nc.vector.memset(mpad3[64:128, 0:47, 48:52].bitcast(F32), 0.0)
            nc.gpsimd.memset(rpad[:], 0.0)
            for i in range(2):
                nc.vector.memset(apad[i][:, 0, :], 0.0)
                nc.vector.memset(apad[i][:, 49:51, :], 0.0)
                nc.vector.memset(apad[i][:, 1:49, 0], 0.0)
                nc.vector.memset(apad[i][:, 1:49, 49], 0.0)

            # ---- phase A: small convs ----
            # match = prelu(w1 @ x + b1) -> both halves of mpad2
            for jb, j0 in enumerate(range(0, 48, 10)):
                nj = min(10, 48 - j0)
                mps = ps_aux.tile([CM, 512], F32, tag="aux")
                nc.tensor.matmul(mps[:, :nj * 48], w1t_sb[:],
                                 x_sb[:, j0 * 48:(j0 + nj) * 48],
                                 start=True, stop=True)
                prelu_evac(mpad2[0:64, 1 + j0:1 + j0 + nj, 1:49],
                           mps[:, :nj * 48], a1, bias=b1_sb[:])
                prelu_evac(mpad2[64:128, j0:j0 + nj, 1:49],
                           mps[:, :nj * 48], a1, bias=b1_sb[:])
                s0 = max(j0, 1)
                cnt = j0 + nj - s0
                so = (s0 - j0) * 48
                prelu_evac(mpad3[0:64, s0 - 1:s0 - 1 + cnt, 1:49],
                           mps[:, so:so + cnt * 48], a1, bias=b1_sb[:])
                prelu_evac(mpad3[64:128, s0 - 1:s0 - 1 + cnt, 0:48],
                           mps[:, so:so + cnt * 48], a1, bias=b1_sb[:])

            # ref = prelu(w2 @ x_sub + b2) -> rpad interior
            nc.vector.tensor_copy(xsub[:], xpad[:, 4:52:3, 4:52:3])
            rps = ps_aux.tile([CM, 512], F32, tag="aux")
            nc.tensor.matmul(rps[:, :L], w2t_sb[:], xsub[:],
                             start=True, stop=True)
            prelu_evac(rpad[:, 1:17, 1:17], rps[:, :L], a2, bias=b2_sb[:])

            # nrm2[l] = sum_{cm,dy,dx} rpad[cm, lh+dy, lw+dx]^2
            sq = work.tile([CM, 18 * 18], F32)
            rpf = rpad[:].rearrange("p a b -> p (a b)")
            nc.vector.tensor_mul(sq[:], rpf, rpf)
            n2ps = ps_aux.tile([1, 512], F32, tag="aux")
            nc.tensor.matmul(n2ps[:, :324], ones64[:], sq[:],
                             start=True, stop=True)
            s2 = small.tile([1, 18, 18], F32)
            nc.vector.tensor_copy(s2[:], n2ps[:, :324].rearrange(
                "p (a b) -> p a b", a=18))
            rs3 = small.tile([1, 18, 16], F32)
            nc.vector.tensor_add(rs3[:], s2[:, :, 0:16], s2[:, :, 1:17])
            nc.vector.tensor_add(rs3[:], rs3[:], s2[:, :, 2:18])
            n2 = small.tile([1, 16, 16], F32)
            nc.vector.tensor_add(n2[:], rs3[:, 0:16, :], rs3[:, 1:17, :])
            nc.vector.tensor_add(n2[:], n2[:], rs3[:, 2:18, :])
            nrm = small.tile([1, L], F32)
            nc.scalar.activation(nrm[:], n2[:].rearrange("p a b -> p (a b)"),
                                 AF.Sqrt, bias=0.0, scale=1.0)
            nc.vector.tensor_scalar_max(out=nrm[:], in0=nrm[:], scalar1=ESC)
            inv = small.tile([1, L], F32)
            nc.vector.reciprocal(inv[:], nrm[:])
            inv10 = small.tile([1, L], F32)
            nc.scalar.mul(inv10[:], inv[:], SM)
            ibps = ps_aux.tile([C, 512], F32, tag="aux")
            nc.tensor.matmul(ibps[:, :L], ones1[:], inv10[:],
                             start=True, stop=True)
            nc.vector.tensor_copy(invb[:], ibps[:, :L])

            # stationary score weights: tap pairs (dy=0,1) stacked on
            # partitions per dx, plus dy=2 singles; columns pre-scaled by
            # 10/max(||patch_l||, eps).
            for dx in range(3):
                nc.vector.tensor_copy(
                    wnp[dx][0:64, :].rearrange("p (a b) -> p a b", a=16),
                    rpad[:, 0:16, dx:dx + 16])
                nc.vector.tensor_copy(
                    wnp[dx][64:128, :].rearrange("p (a b) -> p a b", a=16),
                    rpad[:, 1:17, dx:dx + 16])
                nc.vector.tensor_mul(wnp[dx][:],
                                     wnp[dx][:].bitcast(F32), invb[:])
            nc.vector.tensor_copy(
                wnq[0:64, :].rearrange("p (a b) -> p a b", a=16),
                rpad[:, 2:18, 0:16])
            nc.vector.tensor_copy(
                wnq[64:128, :].rearrange("p (a b) -> p a b", a=16),
                rpad[:, 2:18, 1:17])
            nc.vector.tensor_mul(wnq[:], wnq[:].bitcast(F32), invb[:])
            nc.vector.tensor_copy(
                wns2[:, :].rearrange("p (a b) -> p a b", a=16),
                rpad[:, 2:18, 2:18])
            nc.vector.tensor_mul(wns2[:], wns2[:].bitcast(F32),
                                 invb[0:64, :])

            # ---- phase B: score + max-free softmax, [l, p] layout ----
            m2f = mpad2.rearrange("p a b -> p (a b)")
            m3f = mpad3.rearrange("p a b -> p (a b)")
            pend = []        # deferred (block, score psums) -> exp/sum/scale

            def flush_block(bi, sps_pair):
                for lh in range(2):
                    src = sps_pair[lh][:, :NBN].rearrange(
                        "p (r c) -> p r c", r=NBR)[:, :, :48]
                    # shift logits by -40 so block sums stay inside the
                    # scalar-engine Ln range (softmax is shift-invariant)
                    nc.scalar.activation(
                        esb[lh][:, bi * 384:(bi + 1) * 384].rearrange(
                            "p (r c) -> p r c", r=NBR),
                        src, AF.Exp, bias=negk[:], scale=1.0)
                # partition sums, broadcast to all 128 partitions in one go
                sums = ps_aux.tile([C, 512], F32, tag="aux")
                for lh in range(2):
                    nc.tensor.matmul(sums[:, :384], onesq[:],
                                     esb[lh][:, bi * 384:(bi + 1) * 384],
                                     start=(lh == 0), stop=(lh == 1))
                lns = work.tile([C, 384], F32, tag="lns")
                nc.scalar.activation(lns[:], sums[:, :384], AF.Ln,
                                     bias=0.0, scale=1.0)
                srec = work.tile([C, 384], F32, tag="srec")
                nc.scalar.activation(srec[:], lns[:], AF.Exp,
                                     bias=0.0, scale=-1.0)
                for lh in range(2):
                    dst = apad[lh][:, 1 + bi * NBR:1 + (bi + 1) * NBR, 1:49]
                    nc.vector.tensor_mul(
                        dst,
                        esb[lh][:, bi * 384:(bi + 1) * 384].bitcast(
                            F32).rearrange("p (r c) -> p r c", r=NBR),
                        srec[:].rearrange("p (r c) -> p r c", r=NBR))

            for bi in range(NB):
                j0 = bi * NBR
                sps_pair = []
                for lh in range(2):
                    sps = ps_acc.tile([C, 512], F32, tag="acc")
                    for dx in range(3):
                        nc.tensor.matmul(
                            sps[:, :NBN],
                            wnp[dx][:, lh * 128:lh * 128 + 128],
                            m2f[:, j0 * M2W + dx:j0 * M2W + dx + NBN],
                            start=(dx == 0), stop=False)
                    nc.tensor.matmul(
                        sps[:, :NBN],
                        wnq[:, lh * 128:lh * 128 + 128],
                        m3f[:, j0 * M2W:j0 * M2W + NBN],
                        start=False, stop=False)
                    nc.tensor.matmul(
                        sps[:, :NBN],
                        wns2[:, lh * 128:lh * 128 + 128],
                        m3f[0:64, j0 * M2W + 2:j0 * M2W + 2 + NBN],
                        start=False, stop=True)
                    sps_pair.append(sps)
                pend.append((bi, sps_pair))
                if len(pend) == 2:        # 1-deep software pipeline
                    flush_block(*pend.pop(0))
            while pend:
                flush_block(*pend.pop(0))

            # ---- phase C: dynamic tconv as 9 residue grids ----
            units = [(m, n, ch) for m in range(3) for n in range(3)
                     for ch in range(2)]
            apf = [apad[i].rearrange("c a b -> c (a b)") for i in range(2)]
            for rh in range(3):
                stage = stpool.tile([C, 48, 48, 3], F32)
                for rw in range(3):
                    # contiguous n-shifted residue-(rh,rw) downsample grids:
                    # drn[n][c, a, lw] = xpad[c, 3a+rh, 3(lw+n)+rw]
                    drn = []
                    for n in range(3):
                        d = gpool.tile([C, 18, 16], B16, name=f"drn{n}",
                                       tag=f"drn{n}")
                        nc.vector.tensor_copy(
                            d[:], xpad[:, rh:rh + 52:3,
                                       rw + 3 * n:rw + 3 * n + 46:3])
                        drn.append(d.rearrange("c a b -> c (a b)"))
                    # G production: 18 units -> 5 quads of [128l, 4*128c]
                    quads = []
                    for q in range(5):
                        gps = ps_acc.tile([C, 512], F32, tag="acc")
                        nslot = min(4, 18 - 4 * q)
                        for s in range(nslot):
                            m, n, ch = units[4 * q + s]
                            a0 = ch * 8 + m
                            lhs_ap = drn[n][:, a0 * 16:a0 * 16 + 128]
                            nc.tensor.matmul(
                                gps[:, s * 128:s * 128 + 128],
                                lhs_ap, wat_bf[:],
                                start=True, stop=(not need_ba))
                            if need_ba:
                                nc.tensor.matmul(
                                    gps[:, s * 128:s * 128 + 128],
                                    ones1[:], bar_sb[:],
                                    start=False, stop=True)
                        gsb = gpool.tile([C, 512], mm_dt, tag="gq", bufs=6)
                        prelu_evac(gsb[:, :nslot * 128], gps[:, :nslot * 128],
                                   aa, bias=0.0)
                        quads.append(gsb)
                    # tconv: S[c, ji] accumulation over 18 units; rhs is a
                    # run of the padded attn plane (shift == offset)
                    for jb, j0 in enumerate(range(0, 48, 10)):
                        nj = JBS[jb]
                        vps = ps_acc.tile([C, 512], F32, tag="acc")
                        for u, (m, n, ch) in enumerate(units):
                            lhs = quads[u // 4][:, (u % 4) * 128:
                                                (u % 4) * 128 + 128]
                            if STRIDED_TCONV:
                                rhs = apad[ch][:, j0 + 2 - m:j0 + 2 - m + nj,
                                               2 - n:2 - n + 48]
                                nc.tensor.matmul(vps[:, :nj * 48], lhs, rhs,
                                                 start=(u == 0),
                                                 stop=(u == 17))
                            else:
                                base = (j0 + 2 - m) * APW + (2 - n)
                                nc.tensor.matmul(
                                    vps[:, :nj * APW], lhs,
                                    apf[ch][:, base:base + nj * APW],
                                    start=(u == 0), stop=(u == 17))
                        if STRIDED_TCONV:
                            src = vps[:, :nj * 48].rearrange(
                                "c (j i) -> c j i", j=nj)
                        else:
                            src = vps[:, :nj * APW].rearrange(
                                "c (j i) -> c j i", j=nj)[:, :, :48]
                        dst = stage[:, j0:j0 + nj, :, rw]
                        if (jb + rw) % 2 == 0:
                            nc.scalar.activation(dst, src, AF.Copy,
                                                 bias=0.0, scale=1.0 / 6.0)
                        else:
                            nc.vector.tensor_scalar_mul(
                                out=dst, in0=src, scalar1=1.0 / 6.0)
                        if rw == 2:
                            out_r = out.rearrange(
                                "c (j r) q -> c r j q", r=3)[:, rh]
                            nc.sync.dma_start(
                                out_r[:, j0:j0 + nj, :],
                                stage[:, j0:j0 + nj].rearrange(
                                    "c j i r -> c j (i r)"))
    nc.compile()
    return nc


_CACHE = {}


def _get_program(key):
    if key not in _CACHE:
        _CACHE[key] = build_program(*key)
    return _CACHE[key]


def kernel(x, w1, b1, a1, w2, b2, a2, wa, ba, aa):
    x = np.ascontiguousarray(np.asarray(x, dtype=np.float32))
    w1 = np.asarray(w1, dtype=np.float32)
    w2 = np.asarray(w2, dtype=np.float32)
    wa = np.asarray(wa, dtype=np.float32)
    b1 = np.asarray(b1, dtype=np.float32).reshape(CM, 1)
    b2 = np.asarray(b2, dtype=np.float32).reshape(CM, 1)
    ba = np.asarray(ba, dtype=np.float32).reshape(1, C)
    need_ba = bool(np.any(ba != 0.0))
    key = (float(a1), float(a2), float(aa), need_ba)
    nc = _get_program(key)

    common = {
        "w1t": np.ascontiguousarray(w1.T),
        "w2t": np.ascontiguousarray(w2.T),
        "wat": np.ascontiguousarray(wa.T),
        "b1": b1, "b2": b2, "bar": ba,
    }
    in_maps = [dict(common, x=x[b]) for b in range(B)]
    res = run_bass_kernel_spmd(nc, in_maps, core_ids=list(range(B)))
    return np.stack([res.results[b]["out"] for b in range(B)])


# revision 29
# speedup vs baseline: 1.2099x; 1.2099x over previous
"""CrossScaleAttention Trainium2 kernel.

Full inputs -> full output; shards batch (8 samples) across 8 NeuronCores,
one sample per core (pure data parallel, replicated weights).

Per-core algorithm (restructured from the reference; validated in numpy):
  - bilinear 1/3 downsample with align_corners=False == exact subsample at
    (3i+1, 3j+1).
  - score computed directly in [l, p] layout: stationary operands are the
    normalized ref-patch tap weights (scaled by 10/max(||patch||, eps)),
    moving operand is a 52-wide padded match plane stacked two-tap-deep on
    partitions (dy=0 upper / dy=1 lower halves), so each position-block is
    one contiguous free-dim run. 6 matmuls per (l-half, 8-row block).
  - softmax over l is max-free (empirical |logit| <= ~69 < 88 overflow):
    exp straight out of PSUM, partition-dim sums via ones-matmul, scale by
    a broadcast reciprocal. No PE transposes needed: [l, p] is already the
    layout the deconv wants.
  - dynamic transposed conv decomposes into 9 residue grids (rh, rw); each is
    an accumulation over 18 units (m, n, l-chunk) of matmuls
    S_r[c, ji] += G_{r,m,n}[l, c].T @ attn_shift[l, ji], where
    G_{r,m,n}[l, c] = prelu(gather(x_pad) @ wa^T) is produced directly by the
    TensorEngine from a strided gather of padded x (fusing conv_assembly).
    attn shifts are flat offsets into the 50-wide zero-padded attn plane, so
    the matmul rhs stays a single contiguous free-dim run (N = nj*50).
  - residue planes are interleaved into full output rows in SBUF, DMA'd out.
"""

import sys

for _p in ("/opt/trn_rl_repo", "/root/.axon_site/_ro/trn_rl_repo"):
    if _p not in sys.path:
        sys.path.append(_p)

import numpy as np

import concourse.bass as bass
import concourse.tile as tile
from concourse import bacc, mybir
from concourse.bass_utils import run_bass_kernel_spmd
from concourse.masks import make_identity

F32 = mybir.dt.float32
F32R = mybir.dt.float32r
B16 = mybir.dt.bfloat16
AF = mybir.ActivationFunctionType

# Problem constants (hardcoded per contract)
B, C, H, W = 8, 128, 48, 48
CM = 64
HL = WL = 16
L = 256               # reference patches
SM = 10.0
ESC = 1.0e-4
HP = H + 6            # x padded by 3 -> 54
M2H, M2W = 53, 52     # match plane: 50 padded rows (+3 guard), 52-wide rows
APH, APW = 51, 50     # attn pad: rows 0..49 + guard row, 50-wide
NB = 6                # phase-B position blocks (8 output rows each)
NBR = 8               # rows per phase-B block
NBN = NBR * M2W       # moving-run length per phase-B matmul (416)
JBS = [10, 10, 10, 10, 8]   # j-row blocks for the tconv
STRIDED_TCONV = False  # 48-wide strided tconv rhs (N=480) vs flat 50-wide


def build_program(a1, a2, aa, need_ba, use_prelu=True,
                  mm_dt=F32R, score_dt=F32R):
    """Build the single-core Bass program. Scalars a1/a2/aa baked as imms."""
    nc = bacc.Bacc("TRN2", target_bir_lowering=False, debug=False)

    x = nc.dram_tensor("x", [C, H, W], F32, kind="ExternalInput").ap()
    w1t = nc.dram_tensor("w1t", [C, CM], F32, kind="ExternalInput").ap()
    w2t = nc.dram_tensor("w2t", [C, CM], F32, kind="ExternalInput").ap()
    wat = nc.dram_tensor("wat", [C, C], F32, kind="ExternalInput").ap()
    b1 = nc.dram_tensor("b1", [CM, 1], F32, kind="ExternalInput").ap()
    b2 = nc.dram_tensor("b2", [CM, 1], F32, kind="ExternalInput").ap()
    bar = nc.dram_tensor("bar", [1, C], F32, kind="ExternalInput").ap()
    out = nc.dram_tensor("out", [C, 3 * H, 3 * W], F32, kind="ExternalOutput").ap()

    def prelu_evac(out_ap, in_ap, alpha, bias=0.0):
        nc.scalar.activation(out_ap, in_ap, AF.Prelu,
                             bias=bias, scale=1.0, alpha=float(alpha))

    with tile.TileContext(nc) as tc:
        import contextlib
        ctx = contextlib.ExitStack()
        with ctx:
            consts = ctx.enter_context(tc.tile_pool(name="consts", bufs=1))
            work = ctx.enter_context(tc.tile_pool(name="work", bufs=3))
            small = ctx.enter_context(tc.tile_pool(name="small", bufs=4))
            gpool = ctx.enter_context(tc.tile_pool(name="gpool", bufs=4))
            stpool = ctx.enter_context(tc.tile_pool(name="stage", bufs=2))
            ps_acc = ctx.enter_context(
                tc.tile_pool(name="ps_acc", bufs=4, space="PSUM"))
            ps_aux = ctx.enter_context(
                tc.tile_pool(name="ps_aux", bufs=3, space="PSUM"))

            # ---- constants / inputs in SBUF ----
            w1t_sb = consts.tile([C, CM], F32)
            w2t_sb = consts.tile([C, CM], F32)
            wat_sb = consts.tile([C, C], F32)
            b1_sb = consts.tile([CM, 1], F32)
            b2_sb = consts.tile([CM, 1], F32)
            ones64 = consts.tile([CM, 1], F32)
            ones1 = consts.tile([1, C], F32)
            onesq = consts.tile([C, C], score_dt)
            ident = consts.tile([C, C], F32)
            x_sb = consts.tile([C, H * W], F32)
            xpad = consts.tile([C, HP, HP], F32)
            mpad2 = consts.tile([C, M2H, M2W], score_dt)
            mpad3 = consts.tile([C, M2H, M2W], score_dt)
            rpad = consts.tile([CM, 18, 18], F32)
            xsub = consts.tile([C, L], F32)
            wat_bf = consts.tile([C, C], B16)
            wnp = [consts.tile([C, L], score_dt, name=f"wnp{i}",
                               tag=f"wnp{i}") for i in range(3)]
            wnq = consts.tile([C, L], score_dt)
            wns2 = consts.tile([CM, L], score_dt)
            invb = consts.tile([C, L], F32)
            esb = [consts.tile([C, NB * 8 * 48], score_dt, name=f"esb{i}",
                               tag=f"esb{i}") for i in range(2)]
            apad = [consts.tile([C, APH, APW], mm_dt, name=f"apad{i}",
                                tag=f"apad{i}") for i in range(2)]
            if need_ba:
                bar_sb = consts.tile([1, C], F32)
                nc.sync.dma_start(bar_sb[:], bar)

            nc.sync.dma_start(w1t_sb[:], w1t)
            nc.sync.dma_start(w2t_sb[:], w2t)
            nc.sync.dma_start(wat_sb[:], wat)
            nc.sync.dma_start(b1_sb[:], b1)
            nc.sync.dma_start(b2_sb[:], b2)
            for j0 in range(0, 48, 10):
                nj = min(10, 48 - j0)
                nc.sync.dma_start(
                    x_sb[:, j0 * 48:(j0 + nj) * 48],
                    x.rearrange("c h w -> c (h w)")[:, j0 * 48:(j0 + nj) * 48])
            nc.sync.dma_start(xpad[:, 3:51, 3:51], x)

            nc.gpsimd.memset(ones64[:], 1.0)
            nc.gpsimd.memset(ones1[:], 1.0)
            nc.vector.memset(onesq[:].bitcast(F32), 1.0)
            make_identity(nc, ident[:])

            # HAM warm-up: ~40 throwaway matmuls on the identity while the
            # input DMAs land, so real matmuls start at K=8/8.
            wps = ps_aux.tile([C, 512], F32, tag="aux")
            for wi in range(40):
                nc.tensor.matmul(wps[:, :C], ident[:], ident[:],
                                 start=True, stop=True)

            nc.vector.tensor_copy(wat_bf[:], wat_sb[:])
            # x -> xpad interior; zero borders (pad 3)
            nc.gpsimd.memset(xpad[:, 0:3, :], 0.0)
            nc.gpsimd.memset(xpad[:, 51:54, :], 0.0)
            nc.gpsimd.memset(xpad[:, 3:51, 0:3], 0.0)
            nc.gpsimd.memset(xpad[:, 3:51, 51:54], 0.0)

            # match plane zeros: upper half holds padded rows 0..49, lower
            # half the same plane shifted up one row; cols 0/49 and the
            # 50..51 guard columns are zero, plus guard rows.
            nc.vector.memset(mpad2[0:64, 0, :].bitcast(F32), 0.0)
            nc.vector.memset(mpad2[0:64, 49:53, :].bitcast(F32), 0.0)
            nc.vector.memset(mpad2[0:64, 1:49, 0].bitcast(F32), 0.0)
            nc.vector.memset(mpad2[0:64, 1:49, 49:52].bitcast(F32), 0.0)
            nc.vector.memset(mpad2[64:128, 48:53, :].bitcast(F32), 0.0)
            nc.vector.memset(mpad2[64:128, 0:48, 0].bitcast(F32), 0.0)
            nc.vector.memset(mpad2[64:128, 0:48, 49:52].bitcast(F32), 0.0)
            # mpad3: upper = P shifted up 2 rows; lower = up 2 rows + left
            # 1 col (dy=2 tap pairs for dx=0,1; dx=2 single reads upper+2)
            nc.vector.memset(mpad3[0:64, 47:53, :].bitcast(F32), 0.0)
            nc.vector.memset(mpad3[0:64, 0:47, 0].bitcast(F32), 0.0)
            nc.vector.memset(mpad3[0:64, 0:47, 49:52].bitcast(F32), 0.0)
            nc.vector.memset(mpad3[64:128, 47:53, :].bitcast(F32), 0.0)
            nc.vector.memset(mpad3[64:128, 0:47, 48:52].bitcast(F32), 0.0)
            nc.gpsimd.memset(rpad[:], 0.0)
            for i in range(2):
                nc.vector.memset(apad[i][:, 0, :].bitcast(F32), 0.0)
                nc.vector.memset(apad[i][:, 49:51, :].bitcast(F32), 0.0)
                nc.vector.memset(apad[i][:, 1:49, 0].bitcast(F32), 0.0)
                nc.vector.memset(apad[i][:, 1:49, 49].bitcast(F32), 0.0)

            # ---- phase A: small convs ----
            # match = prelu(w1 @ x + b1) -> both halves of mpad2
            for jb, j0 in enumerate(range(0, 48, 10)):
                nj = min(10, 48 - j0)
                mps = ps_aux.tile([CM, 512], F32, tag="aux")
                nc.tensor.matmul(mps[:, :nj * 48], w1t_sb[:],
                                 x_sb[:, j0 * 48:(j0 + nj) * 48],
                                 start=True, stop=True)
                prelu_evac(mpad2[0:64, 1 + j0:1 + j0 + nj, 1:49],
                           mps[:, :nj * 48], a1, bias=b1_sb[:])
                prelu_evac(mpad2[64:128, j0:j0 + nj, 1:49],
                           mps[:, :nj * 48], a1, bias=b1_sb[:])
                s0 = max(j0, 1)
                cnt = j0 + nj - s0
                so = (s0 - j0) * 48
                prelu_evac(mpad3[0:64, s0 - 1:s0 - 1 + cnt, 1:49],
                           mps[:, so:so + cnt * 48], a1, bias=b1_sb[:])
                prelu_evac(mpad3[64:128, s0 - 1:s0 - 1 + cnt, 0:48],
                           mps[:, so:so + cnt * 48], a1, bias=b1_sb[:])

            # ref = prelu(w2 @ x_sub + b2) -> rpad interior
            nc.vector.tensor_copy(xsub[:], xpad[:, 4:52:3, 4:52:3])
            rps = ps_aux.tile([CM, 512], F32, tag="aux")
            nc.tensor.matmul(rps[:, :L], w2t_sb[:], xsub[:],
                             start=True, stop=True)
            prelu_evac(rpad[:, 1:17, 1:17], rps[:, :L], a2, bias=b2_sb[:])

            # nrm2[l] = sum_{cm,dy,dx} rpad[cm, lh+dy, lw+dx]^2
            sq = work.tile([CM, 18 * 18], F32)
            rpf = rpad[:].rearrange("p a b -> p (a b)")
            nc.vector.tensor_mul(sq[:], rpf, rpf)
            n2ps = ps_aux.tile([1, 512], F32, tag="aux")
            nc.tensor.matmul(n2ps[:, :324], ones64[:], sq[:],
                             start=True, stop=True)
            s2 = small.tile([1, 18, 18], F32)
            nc.vector.tensor_copy(s2[:], n2ps[:, :324].rearrange(
                "p (a b) -> p a b", a=18))
            rs3 = small.tile([1, 18, 16], F32)
            nc.vector.tensor_add(rs3[:], s2[:, :, 0:16], s2[:, :, 1:17])
            nc.vector.tensor_add(rs3[:], rs3[:], s2[:, :, 2:18])
            n2 = small.tile([1, 16, 16], F32)
            nc.vector.tensor_add(n2[:], rs3[:, 0:16, :], rs3[:, 1:17, :])
            nc.vector.tensor_add(n2[:], n2[:], rs3[:, 2:18, :])
            nrm = small.tile([1, L], F32)
            nc.scalar.activation(nrm[:], n2[:].rearrange("p a b -> p (a b)"),
                                 AF.Sqrt, bias=0.0, scale=1.0)
            nc.vector.tensor_scalar_max(out=nrm[:], in0=nrm[:], scalar1=ESC)
            inv = small.tile([1, L], F32)
            nc.vector.reciprocal(inv[:], nrm[:])
            inv10 = small.tile([1, L], F32)
            nc.scalar.mul(inv10[:], inv[:], SM)
            ibps = ps_aux.tile([C, 512], F32, tag="aux")
            nc.tensor.matmul(ibps[:, :L], ones1[:], inv10[:],
                             start=True, stop=True)
            nc.vector.tensor_copy(invb[:], ibps[:, :L])

            # stationary score weights: tap pairs (dy=0,1) stacked on
            # partitions per dx, plus dy=2 singles; columns pre-scaled by
            # 10/max(||patch_l||, eps).
            for dx in range(3):
                nc.vector.tensor_copy(
                    wnp[dx][0:64, :].rearrange("p (a b) -> p a b", a=16),
                    rpad[:, 0:16, dx:dx + 16])
                nc.vector.tensor_copy(
                    wnp[dx][64:128, :].rearrange("p (a b) -> p a b", a=16),
                    rpad[:, 1:17, dx:dx + 16])
                nc.vector.tensor_mul(wnp[dx][:],
                                     wnp[dx][:].bitcast(F32), invb[:])
            nc.vector.tensor_copy(
                wnq[0:64, :].rearrange("p (a b) -> p a b", a=16),
                rpad[:, 2:18, 0:16])
            nc.vector.tensor_copy(
                wnq[64:128, :].rearrange("p (a b) -> p a b", a=16),
                rpad[:, 2:18, 1:17])
            nc.vector.tensor_mul(wnq[:], wnq[:].bitcast(F32), invb[:])
            nc.vector.tensor_copy(
                wns2[:, :].rearrange("p (a b) -> p a b", a=16),
                rpad[:, 2:18, 2:18])
            nc.vector.tensor_mul(wns2[:], wns2[:].bitcast(F32),
                                 invb[0:64, :])

            # ---- phase B: score + max-free softmax, [l, p] layout ----
            m2f = mpad2.rearrange("p a b -> p (a b)")
            m3f = mpad3.rearrange("p a b -> p (a b)")
            pend = []        # deferred (block, score psums) -> exp/sum/scale

            def flush_block(bi, sps_pair):
                for lh in range(2):
                    src = sps_pair[lh][:, :NBN].rearrange(
                        "p (r c) -> p r c", r=NBR)[:, :, :48]
                    nc.scalar.activation(
                        esb[lh][:, bi * 384:(bi + 1) * 384].rearrange(
                            "p (r c) -> p r c", r=NBR),
                        src, AF.Exp, bias=0.0, scale=1.0)
                # partition sums, broadcast to all 128 partitions in one go
                sums = ps_aux.tile([C, 512], F32, tag="aux")
                for lh in range(2):
                    nc.tensor.matmul(sums[:, :384], onesq[:],
                                     esb[lh][:, bi * 384:(bi + 1) * 384],
                                     start=(lh == 0), stop=(lh == 1))
                srec = work.tile([C, 384], F32, tag="srec")
                nc.vector.reciprocal_approx_fast(out=srec[:],
                                                 in_=sums[:, :384])
                for lh in range(2):
                    dst = apad[lh][:, 1 + bi * NBR:1 + (bi + 1) * NBR, 1:49]
                    nc.vector.tensor_mul(
                        dst,
                        esb[lh][:, bi * 384:(bi + 1) * 384].bitcast(
                            F32).rearrange("p (r c) -> p r c", r=NBR),
                        srec[:].rearrange("p (r c) -> p r c", r=NBR))

            for bi in range(NB):
                j0 = bi * NBR
                sps_pair = []
                for lh in range(2):
                    sps = ps_acc.tile([C, 512], F32, tag="acc")
                    for dx in range(3):
                        nc.tensor.matmul(
                            sps[:, :NBN],
                            wnp[dx][:, lh * 128:lh * 128 + 128],
                            m2f[:, j0 * M2W + dx:j0 * M2W + dx + NBN],
                            start=(dx == 0), stop=False)
                    nc.tensor.matmul(
                        sps[:, :NBN],
                        wnq[:, lh * 128:lh * 128 + 128],
                        m3f[:, j0 * M2W:j0 * M2W + NBN],
                        start=False, stop=False)
                    nc.tensor.matmul(
                        sps[:, :NBN],
                        wns2[:, lh * 128:lh * 128 + 128],
                        m3f[0:64, j0 * M2W + 2:j0 * M2W + 2 + NBN],
                        start=False, stop=True)
                    sps_pair.append(sps)
                pend.append((bi, sps_pair))
                if len(pend) == 2:        # 1-deep software pipeline
                    flush_block(*pend.pop(0))
            while pend:
                flush_block(*pend.pop(0))

            # ---- phase C: dynamic tconv as 9 residue grids ----
            units = [(m, n, ch) for m in range(3) for n in range(3)
                     for ch in range(2)]
            apf = [apad[i].rearrange("c a b -> c (a b)") for i in range(2)]
            for rh in range(3):
                stage = stpool.tile([C, 48, 48, 3], F32)
                for rw in range(3):
                    # contiguous n-shifted residue-(rh,rw) downsample grids:
                    # drn[n][c, a, lw] = xpad[c, 3a+rh, 3(lw+n)+rw]
                    drn = []
                    for n in range(3):
                        d = gpool.tile([C, 18, 16], B16, name=f"drn{n}",
                                       tag=f"drn{n}")
                        nc.vector.tensor_copy(
                            d[:], xpad[:, rh:rh + 52:3,
                                       rw + 3 * n:rw + 3 * n + 46:3])
                        drn.append(d.rearrange("c a b -> c (a b)"))
                    # G production: 18 units -> 5 quads of [128l, 4*128c]
                    quads = []
                    for q in range(5):
                        gps = ps_acc.tile([C, 512], F32, tag="acc")
                        nslot = min(4, 18 - 4 * q)
                        for s in range(nslot):
                            m, n, ch = units[4 * q + s]
                            a0 = ch * 8 + m
                            lhs_ap = drn[n][:, a0 * 16:a0 * 16 + 128]
                            nc.tensor.matmul(
                                gps[:, s * 128:s * 128 + 128],
                                lhs_ap, wat_bf[:],
                                start=True, stop=(not need_ba))
                            if need_ba:
                                nc.tensor.matmul(
                                    gps[:, s * 128:s * 128 + 128],
                                    ones1[:], bar_sb[:],
                                    start=False, stop=True)
                        gsb = gpool.tile([C, 512], mm_dt, tag="gq", bufs=6)
                        prelu_evac(gsb[:, :nslot * 128], gps[:, :nslot * 128],
                                   aa, bias=0.0)
                        quads.append(gsb)
                    # tconv: S[c, ji] accumulation over 18 units; rhs is a
                    # run of the padded attn plane (shift == offset)
                    for jb, j0 in enumerate(range(0, 48, 10)):
                        nj = JBS[jb]
                        vps = ps_acc.tile([C, 512], F32, tag="acc")
                        for u, (m, n, ch) in enumerate(units):
                            lhs = quads[u // 4][:, (u % 4) * 128:
                                                (u % 4) * 128 + 128]
                            if STRIDED_TCONV:
                                rhs = apad[ch][:, j0 + 2 - m:j0 + 2 - m + nj,
                                               2 - n:2 - n + 48]
                                nc.tensor.matmul(vps[:, :nj * 48], lhs, rhs,
                                                 start=(u == 0),
                                                 stop=(u == 17))
                            else:
                                base = (j0 + 2 - m) * APW + (2 - n)
                                nc.tensor.matmul(
                                    vps[:, :nj * APW], lhs,
                                    apf[ch][:, base:base + nj * APW],
                                    start=(u == 0), stop=(u == 17))
                        if STRIDED_TCONV:
                            src = vps[:, :nj * 48].rearrange(
                                "c (j i) -> c j i", j=nj)
                        else:
                            src = vps[:, :nj * APW].rearrange(
                                "c (j i) -> c j i", j=nj)[:, :, :48]
                        dst = stage[:, j0:j0 + nj, :, rw]
                        if (jb + rw) % 2 == 0:
                            nc.scalar.activation(dst, src, AF.Copy,
                                                 bias=0.0, scale=1.0 / 6.0)
                        else:
                            nc.vector.tensor_scalar_mul(
                                out=dst, in0=src, scalar1=1.0 / 6.0)
                        if rw == 2:
                            out_r = out.rearrange(
                                "c (j r) q -> c r j q", r=3)[:, rh]
                            nc.sync.dma_start(
                                out_r[:, j0:j0 + nj, :],
                                stage[:, j0:j0 + nj].rearrange(
                                    "c j i r -> c j (i r)"))
    nc.compile()
    return nc


_CACHE = {}


def _get_program(key):
    if key not in _CACHE:
        _CACHE[key] = build_program(*key)
    return _CACHE[key]


def kernel(x, w1, b1, a1, w2, b2, a2, wa, ba, aa):
    x = np.ascontiguousarray(np.asarray(x, dtype=np.float32))
    w1 = np.asarray(w1, dtype=np.float32)
    w2 = np.asarray(w2, dtype=np.float32)
    wa = np.asarray(wa, dtype=np.float32)
    b1 = np.asarray(b1, dtype=np.float32).reshape(CM, 1)
    b2 = np.asarray(b2, dtype=np.float32).reshape(CM, 1)
    ba = np.asarray(ba, dtype=np.float32).reshape(1, C)
    need_ba = bool(np.any(ba != 0.0))
    key = (float(a1), float(a2), float(aa), need_ba)
    nc = _get_program(key)

    common = {
        "w1t": np.ascontiguousarray(w1.T),
        "w2t": np.ascontiguousarray(w2.T),
        "wat": np.ascontiguousarray(wa.T),
        "b1": b1, "b2": b2, "bar": ba,
    }
    in_maps = [dict(common, x=x[b]) for b in range(B)]
    res = run_bass_kernel_spmd(nc, in_maps, core_ids=list(range(B)))
    return np.stack([res.results[b]["out"] for b in range(B)])
